# revision 13
# baseline (speedup 1.0000x reference)
"""Trainium2 Bass kernel: per-token int8 fake-quant x  @  int4-group-dequant W^T.

Math (matches torchao-style reference):
    x_dq = per_token_quant_dequant(x)            # [B*S, I]
    w_dq = (w_int - zeros) * scales per group    # [O, I]
    out  = x_dq @ w_dq.T                         # [B*S, O]

Device factorization:
    x_dq[t, i] = s[t] * qmz[t, i]   with qmz integer in [-255, 255] (exact in fp16)
    out[t, o]  = s[t] * sum_i qmz[t, i] * w_fp16[o, i]

v1 design (this file) vs v0 (246us): transpose-free + fully pipelined.
 - x is shipped twice from host: token-major [T, I] (feeds the per-token
   min/max stats) and contraction-major [I, T] (feeds quant + matmul), so
   the kernel needs NO on-device transposes of the activation stream
   (v0 burned ~38us of PE + ~25us of ACT on 128 PE-transposes).
 - weights arrive host-dequantized as fp16 [I, O] (v0 shipped int8 + a
   16MB expanded-scale tensor and dequantized on DVE/GpSimd; that DMA +
   compute serialized ahead of the first matmul).
 - work is chunked over 256-token slices: stats -> broadcast -> quant ->
   matmul/readout per chunk, so the PE starts after ~1/8 of the quant
   work instead of all of it.
 - per-token quant vectors (1/s and the rounded clip cap), which live as
   [128,1] columns in token-partition space, are moved into the free
   axis of the [I, T] layout by a tiny PE transpose ([128,2] fp32) and a
   partition-broadcast SBUF->SBUF DMA.

Quant chain per token (identical numerics to v0, RNE via +1.5*2^23):
    s = max((max(x,0)-min(x,0))/255, eps); inv = 1/s
    capM = rne(min(x,0)*inv) + 255 + MAGIC
    qmz = min(x*inv + MAGIC, capM) - MAGIC
Engines: GpSimd does x*inv, DVE does (+MAGIC, min capM), ACT does -MAGIC
with the fp16 downcast; ACT also applies s on PSUM readout.

Sharding: data-parallel over tokens, 8 cores x 1024 tokens each.
"""

from contextlib import ExitStack

import numpy as np

import concourse.bass as bass
import concourse.mybir as mybir
import concourse.tile as tile
from concourse import bass_utils
from concourse import masks

FP = mybir.dt.float32
BF = mybir.dt.bfloat16
F16 = mybir.dt.float16
ALU = mybir.AluOpType
ACTF = mybir.ActivationFunctionType

MAGIC = 12582912.0  # 1.5 * 2**23: add/sub forces RNE round-to-integer in fp32
EPS32 = float(np.finfo(np.float32).eps)
GROUP = 32

N_CORES = 8
B, S, D_IN, D_OUT = 4, 2048, 2048, 2048
TOK_FULL = B * S

MAX_WAITS_PER_INST = 1


def split_excess_waits(nc, max_waits=MAX_WAITS_PER_INST):
    """This walrus build rejects instructions with more than one sync-wait
    command. Move excess waits onto same-engine NOPs placed immediately
    before the over-subscribed instruction — semantically identical (the
    engine performs all waits before issuing)."""
    n_split = 0
    for f in nc.m.functions:
        for bb in f.blocks:
            insts = bb.instructions
            if not any(
                i.sync_info is not None and len(i.sync_info.on_wait or []) > max_waits
                for i in insts
            ):
                continue
            new = []
            for inst in insts:
                si = inst.sync_info
                waits = list(si.on_wait) if si is not None and si.on_wait else []
                if len(waits) > max_waits:
                    keep = waits[-max_waits:]
                    rest = waits[: len(waits) - max_waits]
                    for j in range(0, len(rest), max_waits):
                        nop = mybir.InstNoOp(
                            name=f"wsplit_{inst.name}_{j}",
                            engine=inst.engine,
                            ins=[],
                            outs=[],
                            sync_info=mybir.SyncInfo(
                                on_wait=rest[j : j + max_waits], on_update=[]
                            ),
                        )
                        new.append(nop)
                        n_split += 1
                    si.on_wait = keep
                new.append(inst)
            insts[:] = new
    return n_split


def build_nc(tok, d_in, d_out, wdt=F16, nch=4, bcast_dma=False, split_waits=True):
    """Transpose-free pipelined kernel; see module docstring."""
    nt = tok // 128            # token blocks (8)
    ni = d_in // 128           # contraction blocks (16)
    noc = d_out // 512         # psum-wide output chunks (4)
    nchunks = nt // nch        # pipeline chunks (4)
    CW = 128 * nch             # tokens per chunk (256)
    assert tok % (128 * nch) == 0 and d_in % 128 == 0 and d_out % 512 == 0

    nc = bass.Bass("TRN2", target_bir_lowering=False, debug=False)
    x_ti = nc.dram_tensor("x_ti", [tok, d_in], FP, kind="ExternalInput").ap()
    x_it = nc.dram_tensor("x_it", [d_in, tok], FP, kind="ExternalInput").ap()
    wf = nc.dram_tensor("wf", [d_in, d_out], wdt, kind="ExternalInput").ap()
    out = nc.dram_tensor("out", [tok, d_out], F16, kind="ExternalOutput").ap()

    with tile.TileContext(nc) as tc, ExitStack() as ctx:
        const_pool = ctx.enter_context(tc.tile_pool(name="const", bufs=1))
        ident = const_pool.tile([128, 128], FP, tag="ident", name="ident")
        masks.make_identity(nc, ident[:])

        wf_p = ctx.enter_context(tc.tile_pool(name="wfp", bufs=1))
        xti_p = ctx.enter_context(tc.tile_pool(name="xti", bufs=2))
        xq_p = ctx.enter_context(tc.tile_pool(name="xq", bufs=2))
        qx_p = ctx.enter_context(tc.tile_pool(name="qx", bufs=2))
        st_p = ctx.enter_context(tc.tile_pool(name="st", bufs=2))
        row_p = ctx.enter_context(tc.tile_pool(name="row", bufs=2))
        bc_p = ctx.enter_context(tc.tile_pool(name="bc", bufs=2))
        ot_p = ctx.enter_context(tc.tile_pool(name="ot", bufs=2))
        ps_mm = ctx.enter_context(tc.tile_pool(name="psmm", bufs=1, space="PSUM"))
        ps_tr = ctx.enter_context(tc.tile_pool(name="pstr", bufs=1, space="PSUM"))

        # resident dequantized weights, [i, o] contraction-major
        wf_sb = [
            wf_p.tile([128, d_out], wdt, tag=f"wf{i}", name=f"wf{i}")
            for i in range(ni)
        ]

        def emit_w_dma(i):
            nc.scalar.dma_start(wf_sb[i][:], wf[i * 128 : (i + 1) * 128, :])

        for c in range(nchunks):
            base = c * CW
            # ---- input DMAs for this chunk
            xti_t = []
            for j in range(nch):
                xt = xti_p.tile(
                    [128, d_in], FP, tag=f"xti{j}", name=f"xti{c}_{j}", bufs=1
                )
                nc.sync.dma_start(
                    xt[:], x_ti[base + j * 128 : base + (j + 1) * 128, :]
                )
                xti_t.append(xt)
            xq_t = []
            for i in range(ni):
                xq = xq_p.tile([128, CW], FP, tag=f"xq{i}", name=f"xq{c}_{i}")
                nc.sync.dma_start(
                    xq[:], x_it[i * 128 : (i + 1) * 128, base : base + CW]
                )
                xq_t.append(xq)
            # weight DMAs: all emitted in chunk 0 (consumers must follow
            # producers in emission order for tile's dep tracking), but
            # after chunk 0's x DMAs so those win the queue FIFOs
            if c == 0:
                for i in range(ni):
                    emit_w_dma(i)

            # ---- per-token stats (token-partition space)
            s_cols = []
            st2s = []
            for j in range(nch):
                mn = st_p.tile([128, 1], FP, tag=f"mn{j}", name=f"mn{c}_{j}")
                mx = st_p.tile([128, 1], FP, tag=f"mx{j}", name=f"mx{c}_{j}")
                s_t = st_p.tile([128, 1], FP, tag=f"s{j}", name=f"s{c}_{j}")
                u = st_p.tile([128, 1], FP, tag=f"u{j}", name=f"u{c}_{j}")
                st2 = st_p.tile([128, 2], FP, tag=f"st2{j}", name=f"st2{c}_{j}")
                nc.vector.tensor_reduce(
                    mn[:], xti_t[j][:], mybir.AxisListType.X, ALU.min
                )
                nc.vector.tensor_reduce(
                    mx[:], xti_t[j][:], mybir.AxisListType.X, ALU.max
                )
                # mn0 = min(mn, 0);  s = max((max(mx,0) - mn0)/255, eps)
                nc.vector.tensor_scalar(mn[:], mn[:], 0.0, None, ALU.min)
                nc.vector.scalar_tensor_tensor(
                    s_t[:], mx[:], 0.0, mn[:], ALU.max, ALU.subtract
                )
                nc.vector.tensor_scalar(
                    s_t[:], s_t[:], float(np.float32(1.0) / np.float32(255.0)),
                    EPS32, ALU.mult, ALU.max,
                )
                inv = st2[:, 0:1]
                capm = st2[:, 1:2]
                nc.vector.reciprocal(inv, s_t[:])
                # capM = rne(mn0*inv) + MAGIC + 255 (rounding happens at +MAGIC)
                nc.vector.tensor_tensor(u[:], mn[:], inv, ALU.mult)
                nc.vector.tensor_scalar(capm, u[:], MAGIC, 255.0, ALU.add, ALU.add)
                s_cols.append(s_t)
                st2s.append(st2)

            # ---- move (inv, capM) into the free axis and broadcast over
            # partitions: PE-transpose [128,2] -> [2,128], then SBUF->SBUF
            # partition-broadcast DMA into [128, CW] tiles
            rows = row_p.tile([2, CW], FP, tag="rows", name=f"rows{c}")
            for j in range(nch):
                tr = ps_tr.tile([2, 128], FP, tag="tr", name=f"tr{c}_{j}", bufs=1)
                nc.tensor.transpose(tr[:], st2s[j][:], ident[:])
                nc.scalar.copy(rows[:, j * 128 : (j + 1) * 128], tr[:])
            invB = bc_p.tile([128, CW], FP, tag="invB", name=f"invB{c}", bufs=1)
            capB = bc_p.tile([128, CW], FP, tag="capB", name=f"capB{c}", bufs=1)
            # replicate each row into all 128 partitions with a stride-0
            # middle-dim SBUF->SBUF DMA (the DMA re-reads the row per
            # partition; engines cannot read across partitions). Triggered
            # on ACT so the RAW wait on the rows copies is already satisfied
            # by FIFO position and never stalls the trigger engine.
            nc.scalar.dma_start(
                invB[:], rows[0:1, :].unsqueeze(1).to_broadcast((1, 128, CW))
            )
            nc.scalar.dma_start(
                capB[:], rows[1:2, :].unsqueeze(1).to_broadcast((1, 128, CW))
            )

            # ---- quant in contraction-major space:
            # qmz = min(x*inv + MAGIC, capM) - MAGIC   (fp16 out, exact int)
            qx_ts = []
            for i in range(ni):
                qx = qx_p.tile([128, CW], wdt, tag=f"qx{i}", name=f"qx{c}_{i}")
                nc.gpsimd.tensor_tensor(xq_t[i][:], xq_t[i][:], invB[:], ALU.mult)
                nc.vector.scalar_tensor_tensor(
                    xq_t[i][:], xq_t[i][:], MAGIC, capB[:], ALU.add, ALU.min
                )
                nc.scalar.activation(qx[:], xq_t[i][:], ACTF.Copy, bias=-MAGIC)
                qx_ts.append(qx)

            # ---- matmul + scaled readout per token block
            for j in range(nch):
                psums = [
                    ps_mm.tile(
                        [128, 512], FP, tag=f"ps{oc}",
                        name=f"ps{c}_{j}_{oc}", bufs=(1 if oc == noc - 1 else 2),
                    )
                    for oc in range(noc)
                ]
                for i in range(ni):
                    lhsT = qx_ts[i][:, j * 128 : (j + 1) * 128]
                    for oc in range(noc):
                        nc.tensor.matmul(
                            psums[oc][:],
                            lhsT,
                            wf_sb[i][:, oc * 512 : (oc + 1) * 512],
                            start=(i == 0),
                            stop=(i == ni - 1),
                        )
                # readout: the single-buffered last psum first, staged in
                # 1024-wide fp16 tiles (2KB DMA lines); out DMAs trigger on
                # ACT right after the readouts that produce them
                ro_order = list(range(noc - 1, -1, -1))
                for k in range(0, noc, 2):
                    ohi, olo = ro_order[k], ro_order[k + 1]
                    ot = ot_p.tile(
                        [128, 1024], F16, tag="ot", name=f"ot{c}_{j}_{k}", bufs=2
                    )
                    lo = min(ohi, olo)
                    nc.scalar.mul(
                        ot[:, (ohi - lo) * 512 : (ohi - lo + 1) * 512],
                        psums[ohi][:], s_cols[j][:],
                    )
                    nc.scalar.mul(
                        ot[:, (olo - lo) * 512 : (olo - lo + 1) * 512],
                        psums[olo][:], s_cols[j][:],
                    )
                    nc.scalar.dma_start(
                        out[
                            base + j * 128 : base + (j + 1) * 128,
                            lo * 512 : (lo + 2) * 512,
                        ],
                        ot[:],
                    )
    if split_waits:
        split_excess_waits(nc)
    return nc


def _shard_inputs(x, w_int, w_scales, w_zeros, n_cores, wdt_np):
    tok = TOK_FULL // n_cores
    xf = np.ascontiguousarray(x.reshape(TOK_FULL, D_IN).astype(np.float32))
    # host-dequantized weights, transposed to [I, O] contraction-major
    wdq = (
        w_int.astype(np.float32).reshape(D_OUT, D_IN // GROUP, GROUP)
        * w_scales.astype(np.float32)[:, :, None]
    ).reshape(D_OUT, D_IN)
    assert np.all(w_zeros == 0.0), "kernel assumes w_zeros == 0"
    wfT = np.ascontiguousarray(wdq.T.astype(wdt_np))  # [I, O]
    in_maps = []
    for c in range(n_cores):
        xs = xf[c * tok : (c + 1) * tok]
        in_maps.append(
            {
                "x_ti": xs,
                "x_it": np.ascontiguousarray(xs.T),
                "wf": wfT,
            }
        )
    return in_maps


_NC_CACHE = {}


def _get_nc(wdt=F16):
    key = wdt
    if key not in _NC_CACHE:
        _NC_CACHE[key] = build_nc(TOK_FULL // N_CORES, D_IN, D_OUT, wdt=wdt)
    return _NC_CACHE[key]


def _ensure_ntff_hook():
    """This container lacks the antenv.axon_hooks shim that exposes the
    NTFF profile hook; reconstruct it from trn_boot's ctypes path."""
    import sys
    import types

    try:
        from antenv.axon_hooks import get_axon_ntff_profile_hook  # noqa: F401

        return
    except ImportError:
        pass
    hook = None
    try:
        import trn_agent_boot.trn_boot as tb

        hook = tb._ntff_profile_via_ctypes("/opt/axon/libaxon_pjrt.so")
    except Exception:
        hook = None
    mod = types.ModuleType("antenv.axon_hooks")
    mod.get_axon_ntff_profile_hook = lambda: hook
    mod.set_axon_ntff_profile_hook = lambda h: None
    import antenv

    antenv.axon_hooks = mod
    sys.modules["antenv.axon_hooks"] = mod


def kernel(x, w_int, w_scales, w_zeros, _trace=False, _wdt=F16):
    if _trace:
        _ensure_ntff_hook()
    wdt_np = np.float16 if _wdt == F16 else np.dtype("bfloat16") if False else np.float16
    if _wdt == BF:
        import ml_dtypes

        wdt_np = ml_dtypes.bfloat16
    in_maps = _shard_inputs(x, w_int, w_scales, w_zeros, N_CORES, wdt_np)
    nc = _get_nc(_wdt)
    res = bass_utils.run_bass_kernel_spmd(
        nc, in_maps, core_ids=list(range(N_CORES)), trace=_trace
    )
    tok = TOK_FULL // N_CORES
    full = np.concatenate([res.results[c]["out"] for c in range(N_CORES)], axis=0)
    out = full.astype(np.float32).reshape(B, S, D_OUT)
    if _trace:
        return out, res
    return out


# revision 15
# speedup vs baseline: 1.1001x; 1.1001x over previous
"""Trainium2 Bass kernel: per-token int8 fake-quant x  @  int4-group-dequant W^T.

Math (matches torchao-style reference):
    x_dq = per_token_quant_dequant(x)            # [B*S, I]
    w_dq = (w_int - zeros) * scales per group    # [O, I]
    out  = x_dq @ w_dq.T                         # [B*S, O]

Device factorization:
    x_dq[t, i] = s[t] * qmz[t, i]   with qmz integer in [-255, 255] (exact in fp16)
    out[t, o]  = s[t] * sum_i qmz[t, i] * w_fp16[o, i]

v1 design (this file) vs v0 (246us): transpose-free + fully pipelined.
 - x is shipped twice from host: token-major [T, I] (feeds the per-token
   min/max stats) and contraction-major [I, T] (feeds quant + matmul), so
   the kernel needs NO on-device transposes of the activation stream
   (v0 burned ~38us of PE + ~25us of ACT on 128 PE-transposes).
 - weights arrive host-dequantized as fp16 [I, O] (v0 shipped int8 + a
   16MB expanded-scale tensor and dequantized on DVE/GpSimd; that DMA +
   compute serialized ahead of the first matmul).
 - work is chunked over 256-token slices: stats -> broadcast -> quant ->
   matmul/readout per chunk, so the PE starts after ~1/8 of the quant
   work instead of all of it.
 - per-token quant vectors (1/s and the rounded clip cap), which live as
   [128,1] columns in token-partition space, are moved into the free
   axis of the [I, T] layout by a tiny PE transpose ([128,2] fp32) and a
   partition-broadcast SBUF->SBUF DMA.

Quant chain per token (identical numerics to v0, RNE via +1.5*2^23):
    s = max((max(x,0)-min(x,0))/255, eps); inv = 1/s
    capM = rne(min(x,0)*inv) + 255 + MAGIC
    qmz = min(x*inv + MAGIC, capM) - MAGIC
Engines: GpSimd does x*inv, DVE does (+MAGIC, min capM), ACT does -MAGIC
with the fp16 downcast; ACT also applies s on PSUM readout.

Sharding: data-parallel over tokens, 8 cores x 1024 tokens each.
"""

from contextlib import ExitStack

import numpy as np

import concourse.bass as bass
import concourse.mybir as mybir
import concourse.tile as tile
from concourse import bass_utils
from concourse import masks

FP = mybir.dt.float32
BF = mybir.dt.bfloat16
F16 = mybir.dt.float16
ALU = mybir.AluOpType
ACTF = mybir.ActivationFunctionType

MAGIC = 12582912.0  # 1.5 * 2**23: add/sub forces RNE round-to-integer in fp32
EPS32 = float(np.finfo(np.float32).eps)
GROUP = 32

N_CORES = 8
B, S, D_IN, D_OUT = 4, 2048, 2048, 2048
TOK_FULL = B * S

MAX_WAITS_PER_INST = 1


def split_excess_waits(nc, max_waits=MAX_WAITS_PER_INST):
    """This walrus build rejects instructions with more than one sync-wait
    command. Move excess waits onto same-engine NOPs placed immediately
    before the over-subscribed instruction — semantically identical (the
    engine performs all waits before issuing)."""
    n_split = 0
    for f in nc.m.functions:
        for bb in f.blocks:
            insts = bb.instructions
            if not any(
                i.sync_info is not None and len(i.sync_info.on_wait or []) > max_waits
                for i in insts
            ):
                continue
            new = []
            for inst in insts:
                si = inst.sync_info
                waits = list(si.on_wait) if si is not None and si.on_wait else []
                if len(waits) > max_waits:
                    keep = waits[-max_waits:]
                    rest = waits[: len(waits) - max_waits]
                    for j in range(0, len(rest), max_waits):
                        nop = mybir.InstNoOp(
                            name=f"wsplit_{inst.name}_{j}",
                            engine=inst.engine,
                            ins=[],
                            outs=[],
                            sync_info=mybir.SyncInfo(
                                on_wait=rest[j : j + max_waits], on_update=[]
                            ),
                        )
                        new.append(nop)
                        n_split += 1
                    si.on_wait = keep
                new.append(inst)
            insts[:] = new
    return n_split


def build_nc(tok, d_in, d_out, wdt=F16, split_waits=True):
    """Transpose-free software-pipelined kernel; see module docstring.

    Emission is a head/body pipeline over 256-token chunks. The head of
    chunk c+1 (input DMAs, per-token stats, stats transpose, partition
    broadcast) is emitted inside the body of chunk c so every engine's
    FIFO sees its next-chunk work before it would go idle:
      SP   : pure input DMAs only (x, weights) - no compute-gated waits
      GP   : quant P1 (x*inv) + broadcast triggers (empty qPoolDynamic)
      DVE  : stats reduces/chain + quant P2 (+MAGIC, min cap) + P3 (-MAGIC
             with fp16 downcast)
      ACT  : stats-row copies, PSUM readout scaling, output DMA triggers
      PE   : matmuls + tiny [128,2] stats transposes
    """
    CW = 256                   # tokens per pipeline chunk
    nch = CW // 128            # token blocks per chunk (2)
    nchunks = tok // CW        # pipeline chunks (4)
    ni = d_in // 128           # contraction blocks (16)
    noc = d_out // 512         # psum-wide output chunks (4)
    assert tok % CW == 0 and d_in % 128 == 0 and d_out % 512 == 0
    PW = CW * 2 if tok >= CW * 2 else CW   # xq DMA pair width (2KB rows)
    npc = PW // CW             # chunks per xq DMA pair

    nc = bass.Bass("TRN2", target_bir_lowering=False, debug=False)
    x_ti = nc.dram_tensor("x_ti", [tok, d_in], FP, kind="ExternalInput").ap()
    x_it = nc.dram_tensor("x_it", [d_in, tok], FP, kind="ExternalInput").ap()
    wf = nc.dram_tensor("wf", [d_in, d_out], wdt, kind="ExternalInput").ap()
    out = nc.dram_tensor("out", [tok, d_out], F16, kind="ExternalOutput").ap()

    with tile.TileContext(nc) as tc, ExitStack() as ctx:
        const_pool = ctx.enter_context(tc.tile_pool(name="const", bufs=1))
        ident = const_pool.tile([128, 128], FP, tag="ident", name="ident")
        masks.make_identity(nc, ident[:])

        wf_p = ctx.enter_context(tc.tile_pool(name="wfp", bufs=1))
        xti_p = ctx.enter_context(tc.tile_pool(name="xti", bufs=2))
        xq_p = ctx.enter_context(tc.tile_pool(name="xq", bufs=2))
        qx_p = ctx.enter_context(tc.tile_pool(name="qx", bufs=2))
        st_p = ctx.enter_context(tc.tile_pool(name="st", bufs=2))
        row_p = ctx.enter_context(tc.tile_pool(name="row", bufs=2))
        bc_p = ctx.enter_context(tc.tile_pool(name="bc", bufs=1))
        ot_p = ctx.enter_context(tc.tile_pool(name="ot", bufs=2))
        ps_mm = ctx.enter_context(tc.tile_pool(name="psmm", bufs=1, space="PSUM"))
        ps_tr = ctx.enter_context(tc.tile_pool(name="pstr", bufs=1, space="PSUM"))

        wf_sb = [
            wf_p.tile([128, d_out], wdt, tag=f"wf{i}", name=f"wf{i}")
            for i in range(ni)
        ]

        xq_pairs = {}   # pair index -> list of 16 [128, PW] tiles
        state = {}      # chunk -> dict(s_cols, st2s, invB, capB, qx)

        def emit_x_dmas(c):
            for j in range(nch):
                xt = xti_p.tile(
                    [128, d_in], FP, tag=f"xti{j}", name=f"xti{c}_{j}"
                )
                nc.sync.dma_start(
                    xt[:], x_ti[c * CW + j * 128 : c * CW + (j + 1) * 128, :]
                )
                state[c]["xti"].append(xt)
            p = c // npc
            if c % npc == 0:
                tiles = []
                for i in range(ni):
                    xq = xq_p.tile([128, PW], FP, tag=f"xq{i}", name=f"xqp{p}_{i}")
                    nc.sync.dma_start(
                        xq[:], x_it[i * 128 : (i + 1) * 128, p * PW : (p + 1) * PW]
                    )
                    tiles.append(xq)
                xq_pairs[p] = tiles

        def emit_stats(c):
            st = state[c]
            for j in range(nch):
                mn = st_p.tile([128, 1], FP, tag=f"mn{j}", name=f"mn{c}_{j}")
                mx = st_p.tile([128, 1], FP, tag=f"mx{j}", name=f"mx{c}_{j}")
                s_t = st_p.tile([128, 1], FP, tag=f"s{j}", name=f"s{c}_{j}")
                u = st_p.tile([128, 1], FP, tag=f"u{j}", name=f"u{c}_{j}")
                st2 = st_p.tile([128, 2], FP, tag=f"st2{j}", name=f"st2{c}_{j}")
                xt = st["xti"][j]
                nc.vector.tensor_reduce(
                    mn[:], xt[:], mybir.AxisListType.X, ALU.min
                )
                nc.vector.tensor_reduce(
                    mx[:], xt[:], mybir.AxisListType.X, ALU.max
                )
                nc.vector.tensor_scalar(mn[:], mn[:], 0.0, None, ALU.min)
                nc.vector.scalar_tensor_tensor(
                    s_t[:], mx[:], 0.0, mn[:], ALU.max, ALU.subtract
                )
                nc.vector.tensor_scalar(
                    s_t[:], s_t[:], float(np.float32(1.0) / np.float32(255.0)),
                    EPS32, ALU.mult, ALU.max,
                )
                inv = st2[:, 0:1]
                capm = st2[:, 1:2]
                nc.vector.reciprocal(inv, s_t[:])
                nc.vector.tensor_tensor(u[:], mn[:], inv, ALU.mult)
                nc.vector.tensor_scalar(capm, u[:], MAGIC, 255.0, ALU.add, ALU.add)
                st["s_cols"].append(s_t)
                st["st2s"].append(st2)

        def emit_bcast(c):
            st = state[c]
            rows = row_p.tile([2, CW], FP, tag="rows", name=f"rows{c}")
            for j in range(nch):
                tr = ps_tr.tile([2, 128], FP, tag="tr", name=f"tr{c}_{j}", bufs=1)
                nc.tensor.transpose(tr[:], st["st2s"][j][:], ident[:])
                nc.scalar.copy(rows[:, j * 128 : (j + 1) * 128], tr[:])
            invB = bc_p.tile([128, CW], FP, tag="invB", name=f"invB{c}", bufs=1)
            capB = bc_p.tile([128, CW], FP, tag="capB", name=f"capB{c}", bufs=1)
            # stride-0 row replication on the (otherwise idle) gpsimd DMA
            # queue: low latency even when the bulk queues are saturated
            nc.gpsimd.dma_start(
                invB[:], rows[0:1, :].unsqueeze(1).to_broadcast((1, 128, CW))
            )
            nc.gpsimd.dma_start(
                capB[:], rows[1:2, :].unsqueeze(1).to_broadcast((1, 128, CW))
            )
            st["invB"], st["capB"] = invB, capB

        def emit_quant(c):
            st = state[c]
            off = (c % npc) * CW
            for i in range(ni):
                xq = xq_pairs[c // npc][i][:, off : off + CW]
                qx = qx_p.tile([128, CW], wdt, tag=f"qx{i}", name=f"qx{c}_{i}")
                nc.gpsimd.tensor_tensor(xq, xq, st["invB"][:], ALU.mult)
                nc.vector.scalar_tensor_tensor(
                    xq, xq, MAGIC, st["capB"][:], ALU.add, ALU.min
                )
                nc.vector.tensor_scalar(qx[:], xq, MAGIC, None, ALU.subtract)
                st["qx"].append(qx)

        def emit_mm(c, j):
            st = state[c]
            psums = [
                ps_mm.tile(
                    [128, 512], FP, tag=f"ps{oc}",
                    name=f"ps{c}_{j}_{oc}", bufs=(1 if oc == noc - 1 else 2),
                )
                for oc in range(noc)
            ]
            for i in range(ni):
                lhsT = st["qx"][i][:, j * 128 : (j + 1) * 128]
                for oc in range(noc):
                    nc.tensor.matmul(
                        psums[oc][:],
                        lhsT,
                        wf_sb[i][:, oc * 512 : (oc + 1) * 512],
                        start=(i == 0),
                        stop=(i == ni - 1),
                    )
            st["psums"][j] = psums

        def emit_readout(c, j):
            st = state[c]
            psums = st["psums"][j]
            ro_order = list(range(noc - 1, -1, -1))
            for k in range(0, noc, 2):
                ohi, olo = ro_order[k], ro_order[k + 1]
                lo = min(ohi, olo)
                ot = ot_p.tile(
                    [128, 1024], F16, tag="ot", name=f"ot{c}_{j}_{k}", bufs=2
                )
                nc.scalar.mul(
                    ot[:, (ohi - lo) * 512 : (ohi - lo + 1) * 512],
                    psums[ohi][:], st["s_cols"][j][:],
                )
                nc.scalar.mul(
                    ot[:, (olo - lo) * 512 : (olo - lo + 1) * 512],
                    psums[olo][:], st["s_cols"][j][:],
                )
                nc.scalar.dma_start(
                    out[
                        c * CW + j * 128 : c * CW + (j + 1) * 128,
                        lo * 512 : (lo + 2) * 512,
                    ],
                    ot[:],
                )

        def new_state(c):
            state[c] = dict(
                xti=[], s_cols=[], st2s=[], qx=[], psums={}, invB=None, capB=None
            )

        # ---- pipeline head for chunk 0
        new_state(0)
        emit_x_dmas(0)
        for i in range(ni):
            nc.sync.dma_start(wf_sb[i][:], wf[i * 128 : (i + 1) * 128, :])
        emit_stats(0)
        emit_bcast(0)

        # ---- body pipeline
        for c in range(nchunks):
            nxt = c + 1
            if nxt < nchunks:
                new_state(nxt)
                emit_x_dmas(nxt)
            emit_quant(c)
            if nxt < nchunks:
                emit_stats(nxt)
            emit_mm(c, 0)
            if nxt < nchunks:
                emit_bcast(nxt)
            for j in range(1, nch):
                emit_mm(c, j)
                emit_readout(c, j - 1)
            emit_readout(c, nch - 1)
            del state[c]
    if split_waits:
        split_excess_waits(nc)
    return nc


def _shard_inputs(x, w_int, w_scales, w_zeros, n_cores, wdt_np):
    tok = TOK_FULL // n_cores
    xf = np.ascontiguousarray(x.reshape(TOK_FULL, D_IN).astype(np.float32))
    # host-dequantized weights, transposed to [I, O] contraction-major
    wdq = (
        w_int.astype(np.float32).reshape(D_OUT, D_IN // GROUP, GROUP)
        * w_scales.astype(np.float32)[:, :, None]
    ).reshape(D_OUT, D_IN)
    assert np.all(w_zeros == 0.0), "kernel assumes w_zeros == 0"
    wfT = np.ascontiguousarray(wdq.T.astype(wdt_np))  # [I, O]
    in_maps = []
    for c in range(n_cores):
        xs = xf[c * tok : (c + 1) * tok]
        in_maps.append(
            {
                "x_ti": xs,
                "x_it": np.ascontiguousarray(xs.T),
                "wf": wfT,
            }
        )
    return in_maps


_NC_CACHE = {}


def _get_nc(wdt=F16):
    key = wdt
    if key not in _NC_CACHE:
        _NC_CACHE[key] = build_nc(TOK_FULL // N_CORES, D_IN, D_OUT, wdt=wdt)
    return _NC_CACHE[key]


def _ensure_ntff_hook():
    """This container lacks the antenv.axon_hooks shim that exposes the
    NTFF profile hook; reconstruct it from trn_boot's ctypes path."""
    import sys
    import types

    try:
        from antenv.axon_hooks import get_axon_ntff_profile_hook  # noqa: F401

        return
    except ImportError:
        pass
    hook = None
    try:
        import trn_agent_boot.trn_boot as tb

        hook = tb._ntff_profile_via_ctypes("/opt/axon/libaxon_pjrt.so")
    except Exception:
        hook = None
    mod = types.ModuleType("antenv.axon_hooks")
    mod.get_axon_ntff_profile_hook = lambda: hook
    mod.set_axon_ntff_profile_hook = lambda h: None
    import antenv

    antenv.axon_hooks = mod
    sys.modules["antenv.axon_hooks"] = mod


def kernel(x, w_int, w_scales, w_zeros, _trace=False, _wdt=F16):
    if _trace:
        _ensure_ntff_hook()
    wdt_np = np.float16 if _wdt == F16 else np.dtype("bfloat16") if False else np.float16
    if _wdt == BF:
        import ml_dtypes

        wdt_np = ml_dtypes.bfloat16
    in_maps = _shard_inputs(x, w_int, w_scales, w_zeros, N_CORES, wdt_np)
    nc = _get_nc(_wdt)
    res = bass_utils.run_bass_kernel_spmd(
        nc, in_maps, core_ids=list(range(N_CORES)), trace=_trace
    )
    tok = TOK_FULL // N_CORES
    full = np.concatenate([res.results[c]["out"] for c in range(N_CORES)], axis=0)
    out = full.astype(np.float32).reshape(B, S, D_OUT)
    if _trace:
        return out, res
    return out
